# revision 5
# baseline (speedup 1.0000x reference)
"""Single-head causal attention with RoPE on 8 TRN2 NeuronCores.

Problem: B=4, T=4096, C=2048, D=128 (fp32 reference).
  q/k/v = x @ W{q,k,v}.T + b ; rope(q), rope(k); causal softmax(q k^T / sqrt(D)) @ v

Sharding: core c -> batch b = c//2, sequence-half h = c%2 with a zig-zag
(load-balanced) split of query rows: h=0 owns 512-row tiles {0,1,6,7}
(rows [0,1024) u [3072,4096)), h=1 owns tiles {2,3,4,5} (rows [1024,3072)).
Both halves do exactly 72 score-tile matmuls + 264 AV matmuls, so the causal
work is perfectly balanced. K/V are computed for the full sequence on both
cores of a pair (duplicated; no collectives needed).

One SPMD graph for all 8 cores; the causal structure difference between the
two halves is handled by a runtime If() on a per-core flag input.

Kernel math (all matmuls in bf16, fp32 PSUM accumulation / softmax):
  - Q^T/K^T/V^T projections with T on the moving dim (N=512) for PE efficiency
  - scores computed transposed, S^T[k, q], so softmax needs no transposes
  - exp without max-subtraction (logits are ~N(0,0.8); exp stays tiny in f32)
  - causal mask = multiplicative {0,1} tile AFTER exp (only 4 static patterns)
  - softmax denominator folded into the AV matmul by appending a ones column
    to V (V_aug [k, 129]); PSUM accumulates [AV | rowsum] in natural [q,d]
    layout, then a per-partition reciprocal-scale normalizes.
"""

import sys

if "/opt/trn_rl_repo" not in sys.path:
    sys.path.insert(0, "/opt/trn_rl_repo")

import numpy as np
import ml_dtypes

import concourse.mybir as mybir
import concourse.tile as tile
from concourse import bacc
from concourse.masks import make_identity
from concourse.bass_utils import run_bass_kernel_spmd

BF16 = mybir.dt.bfloat16
F32 = mybir.dt.float32
P = 128
B, T, C, D = 4, 4096, 2048, 128
CC = C // P          # 16 contraction chunks
TQ = T // 2          # 2048 own query rows per core
NT = T // 512        # 8 sequence tiles
NQ = TQ // 512       # 4 own query tiles
KC = T // P          # 32 key chunks
SCALE = float(D) ** -0.5
ROPE_BASE = 10000.0

# zig-zag query-tile ownership (global 512-row tile indices)
JOBS_H0 = (0, 1, 6, 7)
JOBS_H1 = (2, 3, 4, 5)

_NC_CACHE = None


def _build_nc():
    nc = bacc.Bacc("TRN2", target_bir_lowering=False, debug=False, num_devices=8)

    xT = nc.declare_dram_parameter("xT", [C, T], BF16, isOutput=False)
    xTq = nc.declare_dram_parameter("xTq", [C, TQ], BF16, isOutput=False)
    wqT = nc.declare_dram_parameter("wqT", [C, D], BF16, isOutput=False)
    wkT = nc.declare_dram_parameter("wkT", [C, D], BF16, isOutput=False)
    wvT = nc.declare_dram_parameter("wvT", [C, D], BF16, isOutput=False)
    cosT = nc.declare_dram_parameter("cosT", [D, T], BF16, isOutput=False)
    sinT = nc.declare_dram_parameter("sinT", [D, T], BF16, isOutput=False)
    cosTq = nc.declare_dram_parameter("cosTq", [D, TQ], BF16, isOutput=False)
    sinTq = nc.declare_dram_parameter("sinTq", [D, TQ], BF16, isOutput=False)
    bq = nc.declare_dram_parameter("bq", [D, 1], F32, isOutput=False)
    bk = nc.declare_dram_parameter("bk", [D, 1], F32, isOutput=False)
    bv = nc.declare_dram_parameter("bv", [D, 1], F32, isOutput=False)
    tri = nc.declare_dram_parameter("tri", [4, P, 512], BF16, isOutput=False)
    flag = nc.declare_dram_parameter("flag", [1, 1], mybir.dt.int32, isOutput=False)
    out = nc.declare_dram_parameter("out", [TQ, D], F32, isOutput=True)

    with tile.TileContext(nc) as tc:
        with (
            tc.tile_pool(name="big", bufs=1) as big,
            tc.tile_pool(name="xin", bufs=3) as xin,
            tc.tile_pool(name="work", bufs=4) as work,
            tc.tile_pool(name="outp", bufs=3) as outp,
            tc.tile_pool(name="ps", bufs=2, space="PSUM") as ps,
            tc.tile_pool(name="acc", bufs=1, space="PSUM") as accp,
        ):
            # ---- constants / small inputs ----
            wq_sb = big.tile([P, CC, D], BF16, name="wq_sb")
            wk_sb = big.tile([P, CC, D], BF16, name="wk_sb")
            wv_sb = big.tile([P, CC, D], BF16, name="wv_sb")
            nc.sync.dma_start(wq_sb[:], wqT.rearrange("(cc p) d -> p cc d", p=P))
            nc.sync.dma_start(wk_sb[:], wkT.rearrange("(cc p) d -> p cc d", p=P))
            nc.sync.dma_start(wv_sb[:], wvT.rearrange("(cc p) d -> p cc d", p=P))
            cos_sb = big.tile([P, T], BF16, name="cos_sb")
            sin_sb = big.tile([P, T], BF16, name="sin_sb")
            cosq_sb = big.tile([P, TQ], BF16, name="cosq_sb")
            sinq_sb = big.tile([P, TQ], BF16, name="sinq_sb")
            nc.sync.dma_start(cos_sb[:], cosT[:])
            nc.sync.dma_start(sin_sb[:], sinT[:])
            nc.sync.dma_start(cosq_sb[:], cosTq[:])
            nc.sync.dma_start(sinq_sb[:], sinTq[:])
            bq_sb = big.tile([P, 1], F32, name="bq_sb")
            bk_sb = big.tile([P, 1], F32, name="bk_sb")
            bv_sb = big.tile([P, 1], F32, name="bv_sb")
            nc.sync.dma_start(bq_sb[:], bq[:])
            nc.sync.dma_start(bk_sb[:], bk[:])
            nc.sync.dma_start(bv_sb[:], bv[:])
            tri_sb = big.tile([P, 4, 512], BF16, name="tri_sb")
            nc.sync.dma_start(tri_sb[:], tri.rearrange("j p q -> p j q"))
            flag_sb = big.tile([1, 1], mybir.dt.int32, name="flag_sb")
            nc.sync.dma_start(flag_sb[:], flag[:])
            ident = big.tile([P, P], BF16, name="ident")
            make_identity(nc, ident[:])

            # ---- persistent activations ----
            kTr = big.tile([P, T], BF16, name="kTr")    # pre-rope K^T
            kT = big.tile([P, T], BF16, name="kT")      # rope'd K^T
            ktmp = big.tile([P, T], BF16, name="ktmp")
            vT = big.tile([P, T], BF16, name="vT")      # V^T
            vA = big.tile([P, KC, D + 1], BF16, name="vA")  # V_aug chunks [k,129]
            qTr = big.tile([P, TQ], BF16, name="qTr")
            qT = big.tile([P, TQ], BF16, name="qT")
            qtmp = big.tile([P, TQ], BF16, name="qtmp")

            xT_r = xT.rearrange("(cc p) t -> p cc t", p=P)
            xTq_r = xTq.rearrange("(cc p) t -> p cc t", p=P)

            # ---- K^T / V^T projections (stream x^T per 512-col tile) ----
            for tt in range(NT):
                xt = xin.tile([P, CC, 512], BF16, tag="xin")
                nc.sync.dma_start(xt[:], xT_r[:, :, tt * 512:(tt + 1) * 512])
                ps_k = ps.tile([P, 512], F32, tag="ps")
                for cc in range(CC):
                    nc.tensor.matmul(ps_k[:], wk_sb[:, cc], xt[:, cc],
                                     start=(cc == 0), stop=(cc == CC - 1))
                nc.scalar.copy(kTr[:, tt * 512:(tt + 1) * 512], ps_k[:])
                ps_v = ps.tile([P, 512], F32, tag="ps")
                for cc in range(CC):
                    nc.tensor.matmul(ps_v[:], wv_sb[:, cc], xt[:, cc],
                                     start=(cc == 0), stop=(cc == CC - 1))
                nc.scalar.copy(vT[:, tt * 512:(tt + 1) * 512], ps_v[:])

            # ---- Q^T projection (own rows) ----
            for tq in range(NQ):
                xq = xin.tile([P, CC, 512], BF16, tag="xin")
                nc.sync.dma_start(xq[:], xTq_r[:, :, tq * 512:(tq + 1) * 512])
                ps_q = ps.tile([P, 512], F32, tag="ps")
                for cc in range(CC):
                    nc.tensor.matmul(ps_q[:], wq_sb[:, cc], xq[:, cc],
                                     start=(cc == 0), stop=(cc == CC - 1))
                nc.scalar.copy(qTr[:, tq * 512:(tq + 1) * 512], ps_q[:])

            # ---- biases (zeros in this problem, but cheap) ----
            nc.vector.tensor_scalar_add(kTr[:], kTr[:], bk_sb[:])
            nc.vector.tensor_scalar_add(vT[:], vT[:], bv_sb[:])
            nc.vector.tensor_scalar_add(qTr[:], qTr[:], bq_sb[:])

            # ---- V_aug: transpose V^T -> V chunks + ones column ----
            nc.vector.memset(vA[:, :, D], 1.0)
            for kc in range(KC):
                ps_t = ps.tile([P, P], BF16, tag="pst")
                nc.tensor.transpose(ps_t[:], vT[:, kc * P:(kc + 1) * P], ident[:])
                nc.scalar.copy(vA[:, kc, 0:D], ps_t[:])

            # ---- RoPE: t_rot = t * cos + swap_halves(t) * sin_signed ----
            H = D // 2
            nc.sync.dma_start(ktmp[0:H, :], kTr[H:P, :])
            nc.sync.dma_start(ktmp[H:P, :], kTr[0:H, :])
            nc.vector.tensor_mul(ktmp[:], ktmp[:], sin_sb[:])
            nc.vector.tensor_mul(kT[:], kTr[:], cos_sb[:])
            nc.vector.tensor_add(kT[:], kT[:], ktmp[:])

            nc.sync.dma_start(qtmp[0:H, :], qTr[H:P, :])
            nc.sync.dma_start(qtmp[H:P, :], qTr[0:H, :])
            nc.vector.tensor_mul(qtmp[:], qtmp[:], sinq_sb[:])
            nc.vector.tensor_mul(qT[:], qTr[:], cosq_sb[:])
            nc.vector.tensor_add(qT[:], qT[:], qtmp[:])

            # ---- causal flash attention over own query tiles ----
            def attention(jobs):
                for lj, tj in enumerate(jobs):
                    ql0 = lj * 512             # local column offset in qT
                    kc_max = 4 * (tj + 1)      # causal key-chunk bound
                    accs = [
                        accp.tile([P, D + 1], F32, tag=f"acc{j}",
                                  name=f"acc_{tj}_{j}")
                        for j in range(4)
                    ]
                    for kc in range(kc_max):
                        ps_s = ps.tile([P, 512], F32, tag="ps")
                        nc.tensor.matmul(ps_s[:], kT[:, kc * P:(kc + 1) * P],
                                         qT[:, ql0:ql0 + 512],
                                         start=True, stop=True)
                        ex = work.tile([P, 512], BF16, tag="expP")
                        nc.scalar.activation(ex[:], ps_s[:],
                                             mybir.ActivationFunctionType.Exp,
                                             scale=SCALE)
                        doff = kc - 4 * tj
                        if 0 <= doff < 4:
                            nc.vector.tensor_mul(ex[:], ex[:], tri_sb[:, doff])
                        for j in range(4):
                            kc_max_j = 4 * tj + j + 1
                            if kc < kc_max_j:
                                nc.tensor.matmul(
                                    accs[j][:], ex[:, j * P:(j + 1) * P],
                                    vA[:, kc],
                                    start=(kc == 0), stop=(kc == kc_max_j - 1))
                    for j in range(4):
                        rcp = outp.tile([P, 1], F32, tag="rcp")
                        nc.vector.reciprocal(rcp[:], accs[j][:, D:D + 1])
                        ob = outp.tile([P, D], F32, tag="ob")
                        nc.vector.tensor_scalar_mul(ob[:], accs[j][:, 0:D], rcp[:])
                        nc.sync.dma_start(
                            out[ql0 + j * P: ql0 + (j + 1) * P, :], ob[:])

            fv = nc.values_load(flag_sb[0:1, 0:1].to_broadcast((1, 1)))
            with tc.If(fv < 1) as cmp:
                attention(JOBS_H0)
            with cmp.Else():
                attention(JOBS_H1)

    nc.compile()
    return nc


def _get_nc():
    global _NC_CACHE
    if _NC_CACHE is None:
        _NC_CACHE = _build_nc()
    return _NC_CACHE


def _own_rows(h):
    if h == 0:
        return np.r_[0:1024, 3072:4096]
    return np.r_[1024:3072]


def _prep_in_maps(x, Wq, Wk, Wv, bq, bk, bv):
    x = np.asarray(x, np.float32)
    bf = ml_dtypes.bfloat16

    # rope tables (rotate-half convention), pre-signed sin
    half = D // 2
    inv = 1.0 / (ROPE_BASE ** (np.arange(half, dtype=np.float32) / half))
    ang = np.arange(T, dtype=np.float32)[:, None] * inv[None, :]       # [T, 64]
    cos_full = np.concatenate([np.cos(ang), np.cos(ang)], 1).T         # [128, T]
    sin_full = np.concatenate([-np.sin(ang), np.sin(ang)], 1).T        # [128, T]
    cosT = cos_full.astype(bf)
    sinT = sin_full.astype(bf)

    # 4 diagonal mask patterns: tri[j][k, q] = 1 if k + 128*j <= q
    k_idx = np.arange(P)[:, None]
    q_idx = np.arange(512)[None, :]
    tri = np.stack([(k_idx + P * j <= q_idx) for j in range(4)]).astype(bf)

    wqT = np.ascontiguousarray(np.asarray(Wq, np.float32).T).astype(bf)
    wkT = np.ascontiguousarray(np.asarray(Wk, np.float32).T).astype(bf)
    wvT = np.ascontiguousarray(np.asarray(Wv, np.float32).T).astype(bf)
    bq_a = np.ascontiguousarray(np.asarray(bq, np.float32).reshape(D, 1))
    bk_a = np.ascontiguousarray(np.asarray(bk, np.float32).reshape(D, 1))
    bv_a = np.ascontiguousarray(np.asarray(bv, np.float32).reshape(D, 1))

    xT_cache = {}
    in_maps = []
    for c in range(8):
        b, h = c // 2, c % 2
        if b not in xT_cache:
            xT_cache[b] = np.ascontiguousarray(x[b].T).astype(bf)  # [C, T]
        xT_b = xT_cache[b]
        rows = _own_rows(h)
        in_maps.append({
            "xT": xT_b,
            "xTq": np.ascontiguousarray(xT_b[:, rows]),
            "wqT": wqT, "wkT": wkT, "wvT": wvT,
            "cosT": cosT, "sinT": sinT,
            "cosTq": np.ascontiguousarray(cosT[:, rows]),
            "sinTq": np.ascontiguousarray(sinT[:, rows]),
            "bq": bq_a, "bk": bk_a, "bv": bv_a,
            "tri": tri,
            "flag": np.array([[h]], np.int32),
        })

    return in_maps


def kernel(x, Wq, Wk, Wv, bq, bk, bv):
    nc = _get_nc()
    in_maps = _prep_in_maps(x, Wq, Wk, Wv, bq, bk, bv)
    res = run_bass_kernel_spmd(nc, in_maps, core_ids=list(range(8)))

    out = np.empty((B, T, D), np.float32)
    for c in range(8):
        b, h = c // 2, c % 2
        out[b, _own_rows(h)] = res.results[c]["out"]
    return out


# revision 27
# speedup vs baseline: 710.3213x; 710.3213x over previous
"""Single-head causal attention with RoPE on 8 TRN2 NeuronCores.

Problem: B=4, T=4096, C=2048, D=128 (fp32 reference).
  q/k/v = x @ W{q,k,v}.T + b ; rope(q), rope(k); causal softmax(q k^T / sqrt(D)) @ v

Sharding: core c -> batch b = c//2, sequence-half h = c%2 with a zig-zag
(load-balanced) split of query rows: h=0 owns 512-row tiles {0,1,6,7}
(rows [0,1024) u [3072,4096)), h=1 owns tiles {2,3,4,5} (rows [1024,3072)).
Both halves do exactly 72 score-tile matmuls + 264 AV matmuls, so the causal
work is perfectly balanced. K/V are computed for the full sequence on both
cores of a pair (duplicated; no collectives needed).

One SPMD graph for all 8 cores; the causal structure difference between the
two halves is handled by a runtime If() on a per-core flag input.

Kernel math (all matmuls in bf16, fp32 PSUM accumulation / softmax):
  - Q^T/K^T/V^T projections with T on the moving dim (N=512) for PE efficiency
  - scores computed transposed, S^T[k, q], so softmax needs no transposes
  - exp without max-subtraction (logits are ~N(0,0.8); exp stays tiny in f32)
  - causal mask = multiplicative {0,1} tile AFTER exp (only 4 static patterns)
  - softmax denominator folded into the AV matmul by appending a ones column
    to V (V_aug [k, 129]); PSUM accumulates [AV | rowsum] in natural [q,d]
    layout, then a per-partition reciprocal-scale normalizes.
"""

import sys

if "/opt/trn_rl_repo" not in sys.path:
    sys.path.insert(0, "/opt/trn_rl_repo")

import numpy as np
import ml_dtypes

import concourse.mybir as mybir
import concourse.tile as tile
from concourse import bacc
from concourse.masks import make_identity
from concourse.bass_utils import run_bass_kernel_spmd

BF16 = mybir.dt.bfloat16
F32 = mybir.dt.float32
P = 128
B, T, C, D = 4, 4096, 2048, 128
CC = C // P          # 16 contraction chunks
TQ = T // 2          # 2048 own query rows per core
NT = T // 512        # 8 sequence tiles
NQ = TQ // 512       # 4 own query tiles
KC = T // P          # 32 key chunks
SCALE = float(D) ** -0.5
ROPE_BASE = 10000.0

# zig-zag query-tile ownership (global 512-row tile indices)
JOBS_H0 = (0, 3, 4, 7)
JOBS_H1 = (1, 2, 5, 6)

_NC_CACHE = None


def _build_nc():
    nc = bacc.Bacc("TRN2", target_bir_lowering=False, debug=False, num_devices=8)

    xT = nc.declare_dram_parameter("xT", [C, T], BF16, isOutput=False)
    wqP = nc.declare_dram_parameter("wqP", [P, CC * D], BF16, isOutput=False)
    wkP = nc.declare_dram_parameter("wkP", [P, CC * D], BF16, isOutput=False)
    wvP = nc.declare_dram_parameter("wvP", [P, CC * D], BF16, isOutput=False)
    cosT = nc.declare_dram_parameter("cosT", [D, T], BF16, isOutput=False)
    sinT = nc.declare_dram_parameter("sinT", [D, T], BF16, isOutput=False)
    bq = nc.declare_dram_parameter("bq", [D, 1], F32, isOutput=False)
    bk = nc.declare_dram_parameter("bk", [D, 1], F32, isOutput=False)
    bv = nc.declare_dram_parameter("bv", [D, 1], F32, isOutput=False)
    tri = nc.declare_dram_parameter("tri", [4, P, 512], BF16, isOutput=False)
    flag = nc.declare_dram_parameter("flag", [1, 1], mybir.dt.int32, isOutput=False)
    out = nc.declare_dram_parameter("out", [TQ, D], F32, isOutput=True)

    with tile.TileContext(nc) as tc:
        with (
            tc.tile_pool(name="big", bufs=1) as big,
            tc.tile_pool(name="xin", bufs=5) as xin,
            tc.tile_pool(name="work", bufs=6) as work,
            tc.tile_pool(name="outp", bufs=6) as outp,
            tc.tile_pool(name="ps", bufs=3, space="PSUM") as ps,
            tc.tile_pool(name="acc", bufs=1, space="PSUM") as accp,
        ):
            # ---- constants / small inputs ----
            wq_sb = big.tile([P, CC, D], BF16, name="wq_sb")
            wk_sb = big.tile([P, CC, D], BF16, name="wk_sb")
            wv_sb = big.tile([P, CC, D], BF16, name="wv_sb")
            flag_sb = big.tile([1, 1], mybir.dt.int32, name="flag_sb")
            nc.scalar.dma_start(flag_sb[:], flag[:])
            bq_sb = big.tile([P, 1], F32, name="bq_sb")
            bk_sb = big.tile([P, 1], F32, name="bk_sb")
            bv_sb = big.tile([P, 1], F32, name="bv_sb")

            # ---- persistent activations (per-512-slice tiles so reads
            # depend only on their own slice's writers) ----
            kTs = [big.tile([P, 512], BF16, tag=f"kT{t}", name=f"kT{t}")
                   for t in range(NT)]
            qTs = [big.tile([P, 512], BF16, tag=f"qT{t}", name=f"qT{t}")
                   for t in range(NT)]
            vAs = [big.tile([P, 4, D + 1], BF16, tag=f"vA{t}", name=f"vA{t}")
                   for t in range(NT)]
            cos_sb = big.tile([P, T], BF16, name="cos_sb")
            sin_sb = big.tile([P, T], BF16, name="sin_sb")
            tri_sb = big.tile([P, 4, 512], BF16, name="tri_sb")
            ident = big.tile([P, P], BF16, name="ident")


            xT_r = xT.rearrange("(cc p) t -> p cc t", p=P)
            H = D // 2

            def proj(w_sb, b_sb, xt):
                pp = ps.tile([P, 512], F32, tag="ps")
                for cc in range(CC):
                    nc.tensor.matmul(pp[:], w_sb[:, cc], xt[:, cc],
                                     start=(cc == 0), stop=(cc == CC - 1))
                raw = work.tile([P, 512], BF16, tag="prj")
                nc.vector.tensor_scalar_add(raw[:], pp[:], b_sb[:])
                return raw

            def rope(dst, raw, sl):
                tmp = work.tile([P, 512], BF16, tag="rtmp")
                nc.scalar.dma_start(tmp[0:H, :], raw[H:P, :])
                nc.scalar.dma_start(tmp[H:P, :], raw[0:H, :])
                nc.vector.tensor_mul(tmp[:], tmp[:], sin_sb[:, sl])
                nc.vector.tensor_mul(dst[:], raw[:], cos_sb[:, sl])
                nc.vector.tensor_add(dst[:], dst[:], tmp[:])

            def attention_job(tj, ol0):
                ql0 = tj * 512             # global column offset in qT
                kc_max = 4 * (tj + 1)      # causal key-chunk bound
                accs = [accp.tile([P, D + 1], F32, tag=f"acc{j}",
                                  name=f"acc_{tj}_{j}")[:]
                        for j in range(4)]
                for kc in range(kc_max):
                    ps_s = ps.tile([P, 512], F32, tag="ps")
                    nc.tensor.matmul(ps_s[:],
                                     kTs[kc // 4][:, (kc % 4) * P:
                                                  (kc % 4 + 1) * P],
                                     qTs[tj][:],
                                     start=True, stop=True)
                    ex = work.tile([P, 512], BF16, tag="expP")
                    nc.scalar.activation(ex[:], ps_s[:],
                                         mybir.ActivationFunctionType.Exp,
                                         scale=SCALE)
                    doff = kc - 4 * tj
                    if 0 <= doff < 4:
                        nc.vector.tensor_mul(ex[:], ex[:], tri_sb[:, doff])
                    for j in range(4):
                        kc_max_j = 4 * tj + j + 1
                        if kc < kc_max_j:
                            nc.tensor.matmul(
                                accs[j], ex[:, j * P:(j + 1) * P],
                                vAs[kc // 4][:, kc % 4],
                                start=(kc == 0), stop=(kc == kc_max_j - 1))
                for j in range(4):
                    rcp = outp.tile([P, 1], F32, tag="rcp")
                    nc.vector.reciprocal(rcp[:], accs[j][:, D:D + 1])
                    ob = outp.tile([P, D], F32, tag="ob")
                    nc.vector.tensor_scalar_mul(ob[:], accs[j][:, 0:D], rcp[:])
                    nc.sync.dma_start(
                        out[ol0 + j * P: ol0 + (j + 1) * P, :], ob[:])

            def branch(jobs):
                asc = sorted(jobs)
                # per-branch constant loads (the If-entry barrier waits on all
                # pre-emitted instructions, so keep everything inside)
                nc.scalar.dma_start(wk_sb[:],
                                    wkP.rearrange("p (cc d) -> p cc d", d=D))
                nc.scalar.dma_start(wv_sb[:],
                                    wvP.rearrange("p (cc d) -> p cc d", d=D))
                nc.scalar.dma_start(wq_sb[:],
                                    wqP.rearrange("p (cc d) -> p cc d", d=D))
                nc.scalar.dma_start(bk_sb[:], bk[:])
                nc.scalar.dma_start(bv_sb[:], bv[:])
                nc.scalar.dma_start(bq_sb[:], bq[:])
                nc.scalar.dma_start(cos_sb[:], cosT[:])
                nc.scalar.dma_start(sin_sb[:], sinT[:])
                nc.scalar.dma_start(tri_sb[:], tri.rearrange("j p q -> p j q"))
                make_identity(nc, ident[:])
                for t in range(NT):
                    nc.vector.memset(vAs[t][:, :, D], 1.0)
                # K/V projection + rope + V_aug stream over the full sequence;
                # Q only for this branch's own query tiles.
                for tt in range(NT):
                    sl = slice(tt * 512, (tt + 1) * 512)
                    xt = xin.tile([P, CC, 512], BF16, tag="xin")
                    for g in range(4):
                        nc.sync.dma_start(xt[:, 4 * g:4 * g + 4, :],
                                          xT_r[:, 4 * g:4 * g + 4, sl])
                    kraw = proj(wk_sb, bk_sb, xt)
                    rope(kTs[tt], kraw, sl)
                    if tt in jobs:
                        qraw = proj(wq_sb, bq_sb, xt)
                        rope(qTs[tt], qraw, sl)
                    vraw = proj(wv_sb, bv_sb, xt)
                    for kk in range(4):
                        ps_t = ps.tile([P, P], BF16, tag="ps")
                        nc.tensor.transpose(ps_t[:],
                                            vraw[:, kk * P:(kk + 1) * P],
                                            ident[:])
                        nc.scalar.copy(vAs[tt][:, kk, 0:D], ps_t[:])
                    # interleave attention jobs as their K prefixes complete
                    for tj in jobs:
                        if 4 * (tj + 1) == 4 * (tt + 1) and tj <= tt:
                            pass  # handled below via ready list
                # emit attention jobs ascending so early jobs only depend on
                # early K tiles and overlap the stream tail
                for tj in asc:
                    attention_job(tj, asc.index(tj) * 512)

            fv = nc.values_load(flag_sb[0:1, 0:1].to_broadcast((1, 1)))
            with tc.If(fv < 1) as cmp:
                branch(JOBS_H0)
            with cmp.Else():
                branch(JOBS_H1)

    nc.compile()
    return nc


def _get_nc():
    global _NC_CACHE
    if _NC_CACHE is None:
        _NC_CACHE = _build_nc()
    return _NC_CACHE


def _own_rows(h):
    if h == 0:
        return np.r_[0:512, 1536:2560, 3584:4096]
    return np.r_[512:1536, 2560:3584]


def _prep_in_maps(x, Wq, Wk, Wv, bq, bk, bv):
    x = np.asarray(x, np.float32)
    bf = ml_dtypes.bfloat16

    # rope tables (rotate-half convention), pre-signed sin
    half = D // 2
    inv = 1.0 / (ROPE_BASE ** (np.arange(half, dtype=np.float32) / half))
    ang = np.arange(T, dtype=np.float32)[:, None] * inv[None, :]       # [T, 64]
    cos_full = np.concatenate([np.cos(ang), np.cos(ang)], 1).T         # [128, T]
    sin_full = np.concatenate([-np.sin(ang), np.sin(ang)], 1).T        # [128, T]
    cosT = cos_full.astype(bf)
    sinT = sin_full.astype(bf)

    # 4 diagonal mask patterns: tri[j][k, q] = 1 if k + 128*j <= q
    k_idx = np.arange(P)[:, None]
    q_idx = np.arange(512)[None, :]
    tri = np.stack([(k_idx + P * j <= q_idx) for j in range(4)]).astype(bf)

    def _wP(W):
        # [D, C] -> [C, D] -> [p, cc, d] -> [P, CC*D]
        wT = np.asarray(W, np.float32).T.reshape(CC, P, D).transpose(1, 0, 2)
        return np.ascontiguousarray(wT.reshape(P, CC * D)).astype(bf)

    wqP, wkP, wvP = _wP(Wq), _wP(Wk), _wP(Wv)
    bq_a = np.ascontiguousarray(np.asarray(bq, np.float32).reshape(D, 1))
    bk_a = np.ascontiguousarray(np.asarray(bk, np.float32).reshape(D, 1))
    bv_a = np.ascontiguousarray(np.asarray(bv, np.float32).reshape(D, 1))

    xT_cache = {}
    in_maps = []
    for c in range(8):
        b, h = c // 2, c % 2
        if b not in xT_cache:
            xT_cache[b] = np.ascontiguousarray(x[b].T).astype(bf)  # [C, T]
        xT_b = xT_cache[b]
        in_maps.append({
            "xT": xT_b,
            "wqP": wqP, "wkP": wkP, "wvP": wvP,
            "cosT": cosT, "sinT": sinT,
            "bq": bq_a, "bk": bk_a, "bv": bv_a,
            "tri": tri,
            "flag": np.array([[h]], np.int32),
        })

    return in_maps


def kernel(x, Wq, Wk, Wv, bq, bk, bv):
    nc = _get_nc()
    in_maps = _prep_in_maps(x, Wq, Wk, Wv, bq, bk, bv)
    res = run_bass_kernel_spmd(nc, in_maps, core_ids=list(range(8)))

    out = np.empty((B, T, D), np.float32)
    for c in range(8):
        b, h = c // 2, c % 2
        out[b, _own_rows(h)] = res.results[c]["out"]
    return out
